# revision 55
# baseline (speedup 1.0000x reference)
"""Multi-head attention TRN2 Bass kernel (8 NeuronCores, SPMD), v3.

Problem: B=4, S=1024, E=1024, H=16 heads of dim 64, fp32.
Sharding: core c = (batch c//2, head-group c%2); host sums the two
partial output projections per batch.

The kernel is PE-bound: 448 512-col bf16 matmul units (~121us at the
sustained ~2.0GHz P0 clock), with the scalar-engine exp stream (64
ACTIVATEs, ~80-96us) as co-critical path. v3 scheduling:
  - inputs split across BOTH HWDGE rings (SP + ACT) in 0.25-0.5MB
    chunks ordered by first use, so the first score tile's inputs
    (wk0/xk0 on SP, wq0/xq0 on ACT) land right after the queue preamble
  - one global in-order PE schedule: score-tile stream (1u each) with
    ctx accumulation lagging LAG tiles behind the exp stream, and
    projection/output-projection filler paced by a PE-work budget
    (~2.9u filler per score tile) in 4-matmul chunks
  - normalize copies ctx PSUM to SBUF first, freeing the 2 ctx banks
    immediately so consecutive (pair,sh) calls never stall the PE
  - output projection: st0-3 stream as filler during sh1 attention;
    st4-7 pre-accumulate pairs 0-2 after the last score tile (reusing
    the freed score/ctx/proj PSUM banks) and finish with pair 3 right
    after the final normalize, keeping the tail short
  - qt/kt/vaug/cat are split into per-producer tiles so conservative
    dependency tracking cannot serialize unrelated stages
"""

from contextlib import ExitStack

import ml_dtypes
import numpy as np

import concourse.bacc as bacc
import concourse.mybir as mybir
import concourse.tile as tile
from concourse.bass_utils import run_bass_kernel_spmd

B, S, E, H = 4, 1024, 1024, 16
HD = 64
HPC = 8
NPAIR = 4
NET = 8
NTT = 8
P = 128

F32 = mybir.dt.float32
F32R = mybir.dt.float32r
BF16 = mybir.dt.bfloat16
EXP = mybir.ActivationFunctionType.Exp
SCALE = 1.0 / 8.0
BF = ml_dtypes.bfloat16

LAG = 4             # ctx tiles lag behind the exp stream
ETP_BUFS = 16       # e-tile pool depth (bounds max ctx lag)
FILL_PER_TILE = 3.3  # filler matmul units per score tile (208u/64)


def _emit(nc, tc, ctx, aps):
    (wq_d, wk_d, wv_d, xq_d, xk_d, xv_d, wo_d, swp_d, out_d) = aps

    iw = ctx.enter_context(tc.tile_pool(name="iw", bufs=1))
    ix = ctx.enter_context(tc.tile_pool(name="ix", bufs=1))
    const = ctx.enter_context(tc.tile_pool(name="const", bufs=1))
    etp = ctx.enter_context(tc.tile_pool(name="etp", bufs=ETP_BUFS))
    obp = ctx.enter_context(tc.tile_pool(name="obp", bufs=3))
    obt4 = ctx.enter_context(tc.tile_pool(name="obt4", bufs=4))
    rcp = ctx.enter_context(tc.tile_pool(name="rcp", bufs=6))
    pp_sc = ctx.enter_context(tc.tile_pool(name="pp_sc", bufs=2, space="PSUM"))
    pp_ctx = ctx.enter_context(tc.tile_pool(name="pp_ctx", bufs=2, space="PSUM"))
    pp_mm = ctx.enter_context(tc.tile_pool(name="pp_mm", bufs=2, space="PSUM"))

    # split per producer/consumer so dependency tracking stays precise
    qts = {(p, h): const.tile([P, 512], BF16, name=f"qt{p}{h}")
           for p in range(NPAIR) for h in range(2)}
    kts = {(p, h): const.tile([P, 512], BF16, name=f"kt{p}{h}")
           for p in range(NPAIR) for h in range(2)}
    vau = [const.tile([P, 1024], BF16, name=f"vau{tt}") for tt in range(NTT)]
    catt = {(p, sh): const.tile([P, 512], BF16, name=f"cat{p}{sh}")
            for p in range(NPAIR) for sh in range(2)}
    wo_t = const.tile([P, 4096], BF16, name="wo_t")
    swp_t = const.tile([P, 128], F32R, name="swp_t")

    # ---- input tiles ----
    wqp = [iw.tile([P, 1024], BF16, name=f"wqp{p}") for p in range(NPAIR)]
    wkp = [iw.tile([P, 1024], BF16, name=f"wkp{p}") for p in range(NPAIR)]
    wvt = iw.tile([P, 4096], BF16, name="wvt")
    # the sh0 halves of xq/xk are on the critical path: 2-et chunk tiles
    # so the first projections can chase the DMA stream
    xq0c = [ix.tile([P, 1024], BF16, name=f"xq0c{c}") for c in range(4)]
    xk0c = [ix.tile([P, 1024], BF16, name=f"xk0c{c}") for c in range(4)]
    xqt1 = ix.tile([P, 4096], BF16, name="xqt1")
    xkt1 = ix.tile([P, 4096], BF16, name="xkt1")
    xvc = [ix.tile([P, 2048], BF16, name=f"xvc{c}") for c in range(4)]

    # ---- input DMAs: two HWDGE rings, ordered by first use ----
    # Ring credits are ~4: issues past that block the issuing queue, so
    # the ACT ring gets only the Q-path up front; the rest of its plan is
    # sprinkled between exps (one per 2 score tiles).
    act_pre = [(wqp[0][:, 0:512], wq_d[:, 0:512])] + \
        [(xq0c[c][:], xq_d[:, c * 1024:(c + 1) * 1024]) for c in range(2)] + \
        [(wqp[0][:, 512:1024], wq_d[:, 512:1024])] + \
        [(xq0c[c][:], xq_d[:, c * 1024:(c + 1) * 1024]) for c in range(2, 4)]
    act_deferred = [
        (wvt[:, 0:2048], wv_d[:, 0:2048]),
        (wvt[:, 2048:4096], wv_d[:, 2048:4096]),
        (xvc[1][:], xv_d[:, 2048:4096]),
        (wqp[1][:], wq_d[:, 1024:2048]),
        (xvc[3][:], xv_d[:, 6144:8192]),
        (wqp[2][:], wq_d[:, 2048:3072]),
        (xqt1[:, 0:2048], xq_d[:, 4096:6144]),
        (xqt1[:, 2048:4096], xq_d[:, 6144:8192]),
        (wqp[3][:], wq_d[:, 3072:4096]),
    ]
    sp_plan = [
        (wkp[0][:, 0:512], wk_d[:, 0:512]),
    ] + [(xk0c[c][:], xk_d[:, c * 1024:(c + 1) * 1024]) for c in range(2)] + [
        (wkp[0][:, 512:1024], wk_d[:, 512:1024]),
    ] + [(xk0c[c][:], xk_d[:, c * 1024:(c + 1) * 1024]) for c in range(2, 4)] + [
        (xkt1[:, 0:2048], xk_d[:, 4096:6144]),
        (xkt1[:, 2048:4096], xk_d[:, 6144:8192]),
        (xvc[0][:], xv_d[:, 0:2048]),
        (xvc[2][:], xv_d[:, 4096:6144]),
        (wkp[1][:], wk_d[:, 1024:2048]),
        (wkp[2][:], wk_d[:, 2048:3072]),
        (wkp[3][:], wk_d[:, 3072:4096]),
        (wo_t[:, 0:2048], wo_d[:, 0:2048]),
        (wo_t[:, 2048:4096], wo_d[:, 2048:4096]),
        (swp_t[:], swp_d[:, :]),
    ]
    for dst, src in act_pre:
        nc.scalar.dma_start(out=dst, in_=src)
    for dst, src in sp_plan:
        nc.sync.dma_start(out=dst, in_=src)

    # ones blocks of the V augmentation: per tt block [128, 8 heads x 128]
    for tt in range(NTT):
        v4 = vau[tt][:, :].rearrange("p (j q c) -> p j q c", q=2, c=P)
        nc.gpsimd.memset(v4[:, :, 0, HD:P], 1.0)
        nc.gpsimd.memset(v4[:, :, 1, 0:HD], 1.0)

    # ---------------- op emitters ----------------
    proj_done = set()
    fill_units = [0.0]
    open_grp = [0]

    def qk_chunks(which, p, h, drain=None):
        w = wqp[p] if which == "q" else wkp[p]
        dst = qts[(p, h)] if which == "q" else kts[(p, h)]

        def rhs(et):
            if h == 0:
                xc = xq0c if which == "q" else xk0c
                return xc[et // 2][:, (et % 2) * 512:(et % 2 + 1) * 512]
            x = xqt1 if which == "q" else xkt1
            return x[:, et * 512:(et + 1) * 512]

        state = {}

        def c1():
            state["ps"] = pp_mm.tile([P, 512], F32, name="ps", tag="mm")
            for et in range(4):
                nc.tensor.matmul(
                    state["ps"][:], lhsT=w[:, et * P:(et + 1) * P],
                    rhs=rhs(et), start=(et == 0), stop=False)

        def c2():
            for et in range(4, NET):
                nc.tensor.matmul(
                    state["ps"][:], lhsT=w[:, et * P:(et + 1) * P],
                    rhs=rhs(et), start=False, stop=(et == NET - 1))
            if drain == "scalar":
                nc.scalar.copy(dst[:], state["ps"][:])
            else:
                nc.vector.tensor_copy(dst[:], state["ps"][:])
            proj_done.add((which, p, h))

        return [(4, c1), (4, c2)]

    vproj_done = set()

    def vproj_chunks(tt):
        x = xvc[tt // 2]
        base = (tt % 2) * 1024
        state = {}

        def c1():
            state["ps"] = pp_mm.tile([P, 512], F32, name="psv", tag="mm")
            for et in range(4):
                nc.tensor.matmul(
                    state["ps"][:],
                    lhsT=x[:, base + et * P:base + (et + 1) * P],
                    rhs=wvt[:, et * 512:(et + 1) * 512],
                    start=(et == 0), stop=False)

        def c2():
            for et in range(4, NET):
                nc.tensor.matmul(
                    state["ps"][:],
                    lhsT=x[:, base + et * P:base + (et + 1) * P],
                    rhs=wvt[:, et * 512:(et + 1) * 512],
                    start=False, stop=(et == NET - 1))
            dstt = vau[tt][:, :].rearrange("p (j q c) -> p j q c", q=2, c=P)
            srcv = state["ps"][:].rearrange("p (j q c) -> p j q c", q=2, c=HD)
            nc.vector.tensor_copy(dstt[:, :, 0, 0:HD], srcv[:, :, 0, :])
            nc.vector.tensor_copy(dstt[:, :, 1, HD:P], srcv[:, :, 1, :])
            vproj_done.add(tt)

        return [(4, c1), (4, c2)]

    norm_count = [0]

    def outgrp_chunk(sti, ih):
        # sti in 0..3 (sh0 s-tiles): contract all 4 pairs' cat columns
        def c():
            ps = pp_mm.tile([P, 512], F32, name="po", tag="mm")
            for p4 in range(NPAIR):
                nc.tensor.matmul(
                    ps[:],
                    lhsT=catt[(p4, 0)][:, sti * P:(sti + 1) * P],
                    rhs=wo_t[:, p4 * 1024 + ih * 512:p4 * 1024 + (ih + 1) * 512],
                    start=(p4 == 0), stop=(p4 == NPAIR - 1))
            ob = obp.tile([P, 512], BF16, name="ob", tag="ob")
            nc.vector.tensor_copy(ob[:], ps[:])
            nc.sync.dma_start(
                out=out_d[sti * P:(sti + 1) * P, ih * 512:(ih + 1) * 512],
                in_=ob[:])
        return [(4, c)]

    # attention call order: sh-major. call = sh*4 + p
    CALLS = [(p, 0) for p in range(NPAIR)] + [(p, 1) for p in range(NPAIR)]
    e_tiles = {}

    def sc_exp(i):
        call, tt = divmod(i, NTT)
        p, sh = CALLS[call]
        kth = kts[(p, tt // 4)]
        kcol = (tt % 4) * P
        q = qts[(p, sh)]
        sAB = pp_sc.tile([P, 1024], F32, name="sAB", tag="sc")
        nc.tensor.matmul(sAB[:, 0:512], lhsT=kth[0:HD, kcol:kcol + P],
                         rhs=q[0:HD, :], start=True, stop=True)
        nc.tensor.matmul(sAB[:, 512:1024], lhsT=kth[HD:P, kcol:kcol + P],
                         rhs=q[HD:P, :], start=True, stop=True)
        eAB = etp.tile([P, 1024], BF16, name="eAB", tag="et")
        nc.scalar.activation(eAB[:], sAB[:], EXP, scale=SCALE)
        e_tiles[i] = eAB

    ctx_ps = {}

    def ctx_mm(k):
        call, tt = divmod(k, NTT)
        p, sh = CALLS[call]
        if tt == 0:
            ctx_ps[call] = (
                pp_ctx.tile([P, 512], F32, name="ctxA", tag="ctx"),
                pp_ctx.tile([P, 512], F32, name="ctxB", tag="ctx"),
            )
        ctxA, ctxB = ctx_ps[call]
        eAB = e_tiles.pop(k)
        bA = p * 256
        nc.tensor.matmul(ctxA[:], lhsT=vau[tt][:, bA:bA + P],
                         rhs=eAB[:, 0:512],
                         start=(tt == 0), stop=(tt == NTT - 1))
        nc.tensor.matmul(ctxB[:], lhsT=vau[tt][:, bA + P:bA + 2 * P],
                         rhs=eAB[:, 512:1024],
                         start=(tt == 0), stop=(tt == NTT - 1))
        if tt == NTT - 1:
            normalize(call)

    def normalize(call):
        p, sh = CALLS[call]
        dst = catt[(p, sh)]
        ctxA, ctxB = ctx_ps.pop(call)
        if call == 7:
            # final call is on the output critical path: do the two
            # partition shifts as fp32r swap-matrix matmuls on the (idle)
            # PE instead of SBUF->SBUF DMA round-trips. The fp32r rounding
            # (~FP22) is negligible for ctx/denominator values.
            cB = rcp.tile([P, 512], F32, name="cB", tag="rc")
            nc.vector.tensor_copy(cB[:], ctxB[:])
            cA = rcp.tile([P, 512], F32R, name="cAr", tag="rc")
            nc.vector.tensor_copy(cA[:], ctxA[:])
            rA = rcp.tile([P, 512], F32, name="rA", tag="rc")
            rB = rcp.tile([P, 512], F32, name="rB", tag="rc")
            rBr = rcp.tile([P, 512], F32R, name="rBr", tag="rc")
            nc.vector.reciprocal_approx_fast(rB[0:HD, :], cB[0:HD, :])
            nc.vector.tensor_copy(rBr[0:HD, :], rB[0:HD, :])
            swB = pp_ctx.tile([P, 512], F32, name="swB", tag="ctx")
            nc.tensor.matmul(swB[:], lhsT=swp_t[0:HD, :],
                             rhs=rBr[0:HD, :], start=True, stop=True)
            nc.vector.tensor_mul(dst[HD:P, :], cB[HD:P, :], swB[HD:P, :])
            swA = pp_ctx.tile([P, 512], F32, name="swA", tag="ctx")
            nc.tensor.matmul(swA[:], lhsT=swp_t[HD:P, :],
                             rhs=cA[HD:P, :], start=True, stop=True)
            nc.vector.reciprocal_approx_fast(rA[0:HD, :], swA[0:HD, :])
            nc.vector.tensor_mul(dst[0:HD, :], cA[0:HD, :].bitcast(F32),
                                 rA[0:HD, :])
        else:
            # copy PSUM ctx to SBUF first: frees both ctx banks immediately
            cA = rcp.tile([P, 512], F32, name="cA", tag="rc")
            nc.vector.tensor_copy(cA[:], ctxA[:])
            cB = rcp.tile([P, 512], F32, name="cB", tag="rc")
            nc.vector.tensor_copy(cB[:], ctxB[:])
            rA = rcp.tile([P, 512], F32, name="rA", tag="rc")
            rB = rcp.tile([P, 512], F32, name="rB", tag="rc")
            # A: ctx rows 0:64, denom rows 64:128 (64 identical copies)
            sA = rcp.tile([P, 512], F32, name="sA", tag="rc")
            nc.sync.dma_start(out=sA[0:HD, :], in_=cA[HD:P, :])
            nc.vector.reciprocal_approx_fast(rA[0:HD, :], sA[0:HD, :])
            nc.vector.tensor_mul(dst[0:HD, :], cA[0:HD, :], rA[0:HD, :])
            # B: denom rows 0:64, ctx rows 64:128
            nc.vector.reciprocal_approx_fast(rB[0:HD, :], cB[0:HD, :])
            nc.sync.dma_start(out=rB[HD:P, :], in_=rB[0:HD, :])
            nc.vector.tensor_mul(dst[HD:P, :], cB[HD:P, :], rB[HD:P, :])
        norm_count[0] += 1

    # ---------------- filler machinery ----------------
    def mk(dep, chunks):
        return {"chunks": chunks, "next": 0, "dep": dep}

    always = lambda: True
    fillers = [
        mk(always, qk_chunks("k", 0, 1)),
        mk(always, vproj_chunks(0)),
        mk(always, vproj_chunks(1)),
        mk(always, vproj_chunks(2)),
        mk(always, vproj_chunks(3)),
        mk(always, qk_chunks("k", 1, 0)),
        mk(always, qk_chunks("q", 1, 0)),
        mk(always, vproj_chunks(4)),
        mk(always, vproj_chunks(5)),
        mk(always, vproj_chunks(6)),
        mk(always, vproj_chunks(7)),
        mk(always, qk_chunks("k", 1, 1)),
        mk(always, qk_chunks("k", 2, 0)),
        mk(always, qk_chunks("q", 2, 0)),
        mk(always, qk_chunks("k", 2, 1)),
        mk(always, qk_chunks("q", 0, 1)),
        mk(always, qk_chunks("k", 3, 0)),
        mk(always, qk_chunks("q", 3, 0)),
        mk(always, qk_chunks("k", 3, 1)),
        mk(always, qk_chunks("q", 1, 1)),
        mk(always, qk_chunks("q", 2, 1)),
        mk(always, qk_chunks("q", 3, 1)),
    ]
    qk_order = [("k", 0, 1), None, None, None, None, ("k", 1, 0), ("q", 1, 0),
                None, None, None, None, ("k", 1, 1), ("k", 2, 0), ("q", 2, 0),
                ("k", 2, 1), ("q", 0, 1), ("k", 3, 0), ("q", 3, 0),
                ("k", 3, 1), ("q", 1, 1), ("q", 2, 1), ("q", 3, 1)]
    qk_items = {key: it for it, key in zip(fillers, qk_order) if key}
    vp_order = [None, 0, 1, 2, 3, None, None, 4, 5, 6, 7]
    vp_items = {key: it for it, key in zip(fillers, vp_order)
                if key is not None}
    for sti in range(4):
        for ih in range(2):
            fillers.append(mk(lambda: norm_count[0] >= 4,
                              outgrp_chunk(sti, ih)))

    def emit_item_chunk(it):
        n = it["next"]
        if n == 0:
            open_grp[0] += 1
        u, fn = it["chunks"][n]
        fn()
        it["next"] += 1
        if it["next"] >= len(it["chunks"]):
            open_grp[0] -= 1
        fill_units[0] += u
        return u

    def emit_filler_chunk():
        for it in fillers:
            n = it["next"]
            if n >= len(it["chunks"]):
                continue
            if n == 0 and (open_grp[0] >= 2 or not it["dep"]()):
                continue
            return emit_item_chunk(it)
        return 0

    def finish_item(it):
        # a force-start while 2 groups are open would clobber a PSUM slot:
        # close the open groups first
        if it["next"] == 0 and open_grp[0] >= 2:
            for other in fillers:
                if other is not it and 0 < other["next"] < len(other["chunks"]):
                    while other["next"] < len(other["chunks"]):
                        emit_item_chunk(other)
        while it["next"] < len(it["chunks"]):
            emit_item_chunk(it)

    def ensure_proj(which, p, h):
        if (which, p, h) not in proj_done:
            finish_item(qk_items[(which, p, h)])

    def ensure_vproj(tt):
        if tt not in vproj_done:
            finish_item(vp_items[tt])

    # ---------------- main schedule ----------------
    # Keep-alive dummy matmuls between the prologue chunks: they execute
    # while the PE would otherwise sit in DMA-chase gaps, holding the
    # p-state ramp so the real matmuls run at full clock.
    # the dummy accumulator lives in the score ring (idle until sc0) so the
    # proj ring keeps exactly two live tiles (K00 + Q00) in the prologue
    dummy_in = const.tile([P, 512], BF16, name="dummy_in")
    nc.vector.memset(dummy_in[:], 0.0)
    dummy_ps = pp_sc.tile([P, 1024], F32, name="dummy_ps", tag="sc")

    def dummies(n):
        for _ in range(n):
            nc.tensor.matmul(dummy_ps[:, 0:512], lhsT=dummy_in[:, 0:P],
                             rhs=dummy_in[:], start=True, stop=True)

    # The front block runs from the global start barrier (~+7us) until the
    # first weight DMA lands (~+11us), so the PE enters the real prologue
    # already ramped to full clock.
    dummies(7)
    # K00 drains on the scalar engine so both prologue drains run in
    # parallel (the exp stream hasn't started yet)
    kc = qk_chunks("k", 0, 0, drain="scalar")
    qc = qk_chunks("q", 0, 0)
    # interleave K/Q chunk consumption to match the two rings' parallel
    # chunk arrivals (K chunks on SP, Q chunks on ACT land pairwise)
    kc[0][1]()
    dummies(2)
    qc[0][1]()
    dummies(2)
    kc[1][1]()
    dummies(2)
    qc[1][1]()
    dummies(2)
    proj_done.add(("k", 0, 0))
    proj_done.add(("q", 0, 0))

    ctx_next = [0]

    def emit_ctx_upto(limit):
        while ctx_next[0] <= limit:
            k = ctx_next[0]
            _, tt = divmod(k, NTT)
            ensure_vproj(tt)
            ctx_mm(k)
            ctx_next[0] += 1

    for i in range(64):
        call, tt = divmod(i, NTT)
        p, sh = CALLS[call]
        emit_ctx_upto(i - ETP_BUFS)
        ensure_proj("q", p, sh)
        ensure_proj("k", p, 0)
        if tt >= 4:
            ensure_proj("k", p, 1)
        sc_exp(i)
        # sprinkle deferred ACT-ring input DMAs (one per 2 score tiles, so
        # ring-credit waits never block the exp stream)
        if i % 2 == 0 and act_deferred:
            dst, src = act_deferred.pop(0)
            nc.scalar.dma_start(out=dst, in_=src)
        if i >= LAG:
            emit_ctx_upto(i - LAG)
        # freeze the filler budget near the end: the last score tiles pack
        # tightly so the exp stream finishes earlier; leftover filler runs
        # below, between ctx(62) and ctx(63), hiding the exp(63) wait
        while fill_units[0] < (min(i, 60) + 1) * FILL_PER_TILE:
            if emit_filler_chunk() == 0:
                break

    emit_ctx_upto(62)
    for it in fillers:
        finish_item(it)
    # t7's PSUM banks (the proj/outgrp slots) are free already: its
    # pre-accumulation fills the exp(63) wait window
    t7 = (pp_mm.tile([P, 512], F32, name="t7a", tag="mm"),
          pp_mm.tile([P, 512], F32, name="t7b", tag="mm"))
    for p4 in range(3):
        for ih in range(2):
            nc.tensor.matmul(
                t7[ih][:],
                lhsT=catt[(p4, 1)][:, 3 * P:4 * P],
                rhs=wo_t[:, p4 * 1024 + ih * 512:p4 * 1024 + (ih + 1) * 512],
                start=(p4 == 0), stop=False)
    emit_ctx_upto(63)

    # ---- tail: st4..7 output projections ----
    # all 8 (st, ih) groups pre-accumulate pairs 0..2 in the freed PSUM
    # banks; after the final normalize only 8 single-unit matmuls remain.
    # Drains alternate DVE/GpSimd; out-DMAs round-robin 3 queues.
    tg4 = pp_sc.tile([P, 1024], F32, name="tg4", tag="sc")
    tg5 = pp_sc.tile([P, 1024], F32, name="tg5", tag="sc")
    t6 = (pp_ctx.tile([P, 512], F32, name="t6a", tag="ctx"),
          pp_ctx.tile([P, 512], F32, name="t6b", tag="ctx"))
    groups = [(4, 0, tg4[:, 0:512]), (4, 1, tg4[:, 512:1024]),
              (5, 0, tg5[:, 0:512]), (5, 1, tg5[:, 512:1024]),
              (6, 0, t6[0][:]), (6, 1, t6[1][:])]

    def tail_mm(dst, sti, ih, p4, start, stop):
        nc.tensor.matmul(
            dst,
            lhsT=catt[(p4, 1)][:, (sti - 4) * P:(sti - 3) * P],
            rhs=wo_t[:, p4 * 1024 + ih * 512:p4 * 1024 + (ih + 1) * 512],
            start=start, stop=stop)

    for p4 in range(3):
        for sti, ih, dst in groups:
            tail_mm(dst, sti, ih, p4, start=(p4 == 0), stop=False)
    groups += [(7, 0, t7[0][:]), (7, 1, t7[1][:])]
    for sti, ih, dst in groups:
        tail_mm(dst, sti, ih, 3, start=False, stop=True)
    # drains: tg4/tg5 as single wide copies on DVE/ACT in parallel, then
    # the four [128,512] halves alternating engines; out-DMA issues go on
    # the sync + gpsimd(SWDGE) queues (the scalar queue runs the copies)
    # keep gpsimd out of the tail entirely: its end-of-queue DRAIN is slow
    # (~2.5us) and would gate the final sweep. Scalar-copied groups store
    # on the scalar ring itself (no cross-queue hop); DVE-copied ones on
    # sync. DVE/ACT drains alternate so they run in parallel.
    ob4 = obp.tile([P, 1024], BF16, name="ob4", tag="obw")
    nc.vector.tensor_copy(ob4[:], tg4[:])
    nc.sync.dma_start(out=out_d[4 * P:5 * P, :], in_=ob4[:])
    ob5 = obp.tile([P, 1024], BF16, name="ob5", tag="obw")
    nc.scalar.copy(ob5[:], tg5[:])
    nc.scalar.dma_start(out=out_d[5 * P:6 * P, :], in_=ob5[:])
    small = [(6, 0, t6[0], nc.vector.tensor_copy, nc.sync),
             (6, 1, t6[1], nc.scalar.copy, nc.scalar),
             (7, 0, t7[0], nc.vector.tensor_copy, nc.sync),
             (7, 1, t7[1], nc.scalar.copy, nc.scalar)]
    for sti, ih, src, cp, q in small:
        ob = obt4.tile([P, 512], BF16, name="obt", tag="obt")
        cp(ob[:], src[:])
        q.dma_start(
            out=out_d[sti * P:(sti + 1) * P, ih * 512:(ih + 1) * 512],
            in_=ob[:])


_CACHE = {}


def build():
    if "nc" in _CACHE:
        return _CACHE["nc"]
    nc = bacc.Bacc("TRN2", target_bir_lowering=False, debug=False)
    wq_d = nc.dram_tensor("wq", [P, 4096], BF16, kind="ExternalInput").ap()
    wk_d = nc.dram_tensor("wk", [P, 4096], BF16, kind="ExternalInput").ap()
    wv_d = nc.dram_tensor("wv", [P, 4096], BF16, kind="ExternalInput").ap()
    xq_d = nc.dram_tensor("xq", [P, 8192], BF16, kind="ExternalInput").ap()
    xk_d = nc.dram_tensor("xk", [P, 8192], BF16, kind="ExternalInput").ap()
    xv_d = nc.dram_tensor("xv", [P, 8192], BF16, kind="ExternalInput").ap()
    wo_d = nc.dram_tensor("wo", [P, 4096], BF16, kind="ExternalInput").ap()
    swp_d = nc.dram_tensor("swp", [P, 128], F32R, kind="ExternalInput").ap()
    out_d = nc.dram_tensor("out", [S, E], BF16, kind="ExternalOutput").ap()
    with tile.TileContext(nc) as tc, ExitStack() as ctx:
        _emit(nc, tc, ctx,
              (wq_d, wk_d, wv_d, xq_d, xk_d, xv_d, wo_d, swp_d, out_d))
    nc.compile()
    _CACHE["nc"] = nc
    return nc


def make_in_maps(query, key, value, Wq, Wk, Wv, Wo):
    in_maps = []
    for c in range(8):
        b, g = divmod(c, 2)
        hs = slice(g * HPC, (g + 1) * HPC)

        def bf(a):
            return np.ascontiguousarray(a, dtype=np.float32).astype(BF)

        # per-pair Q/K weights: w[ep, p*1024 + et*128 + hp*64 + d]
        def wqk(Wfull):
            A = np.asarray(Wfull[hs], np.float32)          # [8, 64, 1024]
            Ap = A.reshape(NPAIR, 2, HD, NET, P)           # [p, hp, d, et, ep]
            Ap = Ap.transpose(4, 0, 3, 1, 2)               # [ep, p, et, hp, d]
            return bf(Ap.reshape(P, 4096))

        # x for Q/K: x[ep, h*4096 + et*512 + s]
        def xqk(x):
            X = np.asarray(x, np.float32)                  # [1024 s, 1024 e]
            Xp = X.reshape(2, 512, NET, P)                 # [h, s, et, ep]
            Xp = Xp.transpose(3, 0, 2, 1)                  # [ep, h, et, s]
            return bf(Xp.reshape(P, 8192))

        # x for V: x[ep, tt*1024 + et*128 + tl]
        V = np.asarray(value[b], np.float32)
        Vp = V.reshape(NTT, P, NET, P).transpose(3, 0, 2, 1)
        xv = bf(Vp.reshape(P, 8192))

        # wv: [ep, et*512 + h*64 + d]
        Bv = np.asarray(Wv[hs], np.float32)
        Bp = Bv.reshape(HPC, HD, NET, P).transpose(3, 2, 0, 1)
        wv = bf(Bp.reshape(P, 4096))

        # wo: [f, p4*1024 + i]
        C = np.asarray(Wo[:, g * 512:(g + 1) * 512], np.float32)
        Cp = C.reshape(E, NPAIR, P).transpose(2, 1, 0)
        wo = bf(Cp.reshape(P, 4096))

        swp = np.zeros((P, P), np.float32)
        swp[np.arange(P), (np.arange(P) + HD) % P] = 1.0

        in_maps.append({
            "wq": wqk(Wq), "wk": wqk(Wk), "wv": wv,
            "xq": xqk(query[b]), "xk": xqk(key[b]), "xv": xv,
            "wo": wo, "swp": swp,
        })
    return in_maps


def kernel(query, key, value, Wq, Wk, Wv, Wo):
    nc = build()
    in_maps = make_in_maps(query, key, value, Wq, Wk, Wv, Wo)
    res = run_bass_kernel_spmd(nc, in_maps, list(range(8))).results
    out = np.empty((B, S, E), np.float32)
    for b in range(B):
        out[b] = (np.asarray(res[2 * b]["out"], np.float32)
                  + np.asarray(res[2 * b + 1]["out"], np.float32))
    return out
